# revision 15
# baseline (speedup 1.0000x reference)
"""AriaGroupedGEMM (MoE grouped GEMM) on 8 TRN2 NeuronCores.

Problem: input [4096, 2048] f32, weight [8, 2048, 2048] f32,
tokens_per_expert [8] int32 (tokens pre-sorted by expert).
out[i] = input[i] @ weight[expert_of(i)].

Strategy: expert-parallel. Core g owns expert g's weight and its token
group and runs a dense [T_pad, 2048] @ [2048, 2048] GEMM.

Precision: w is fp8 e3m4 pre-scaled by 64 on host (lifts ~N(0,0.02)
into e3m4's normal range); x is bf16 at natural scale for k 512..2047
and fp8 e3m4 (natural scale) for k 0..511, so every product is 64*x*w
and the PSUM->SBUF cast multiplies by 1/64. The PE's fp22 datapath
reproduces e3m4/bf16 products exactly (verified on HW vs exact
quantized arithmetic: 2e-8). Norm rel-err ~1.51e-2, under the 2e-2
gate. fp8 weights + fp8 first-quarter x shrink the first-need DMA
window to ~768KB and halve total weight traffic, so real matmuls can
start cold at ~8.7us and never starve.

Schedule (trace-driven):
- ~7.1us fixed engine-init preamble (runtime start gate + instruction
  load + prologue barrier) precedes everything; nothing can issue before
  it ends.
- Pre-TileContext warmup matmuls (raw, dep-free, garbage SBUF inputs)
  start right at the preamble end and keep the PE busy until real data
  lands, so the HAM clock-gate un-throttles to 2.4GHz ~3.4us later and
  real matmuls never run at the 1.2GHz cold rate for long.
- Input DMAs are issued in exact first-need order, the first 8
  alternating the two HWDGE rings (sync + scalar), the bulk on sync.
  w tiles are FULL k-rows [P, 2048] so phase B (right half) needs no
  additional DMA data at all.
- Phase A: left 1024 columns for all m-tiles, ko-outer m-inner (each
  fresh w row amortizes across 4 m-tiles -> compute-bound after round
  0). All 8 PSUM banks: 4 m x 2 halves. Per-m casts fire right after
  that m's ko=15 matmuls.
- Phase B: right 1024 columns, m-outer, reusing w from SBUF. The final
  m-tile computes its last 512 columns as two 256-wide groups so only a
  tiny cast+DMA trails the last matmul.
- All PSUM->SBUF casts run on the Vector engine (DVE does 16-bit output
  at 2 elem/cycle and never collides with the Scalar engine's DMA issue
  stream); output DMAs are issued from Scalar, the final one from Sync.
"""
import sys
import functools

for _p in ("/opt/trn_rl_repo", "/root/.axon_site/_ro/trn_rl_repo"):
    if _p not in sys.path:
        sys.path.insert(0, _p)

import numpy as np
import ml_dtypes

import concourse.mybir as mybir
import concourse.tile as tile
from concourse import bacc
from concourse import bass_utils

P = 128
K = 2048            # in_features (contraction)
N = 2048            # out_features
G = 8               # experts == cores
KO = K // P         # 16 k-subtiles
HALF = N // 2

X_DT = mybir.dt.bfloat16
W_DT = mybir.dt.float8e3
OUT_DT = mybir.dt.bfloat16      # psum(f32) -> bf16 on the way out; host upcasts
NP_X = ml_dtypes.bfloat16
NP_W = ml_dtypes.float8_e3m4

WSCALE = 64.0       # w*64 into e3m4; x stays natural scale; every psum
                    # value is 64*(x@w) and the cast scales by 1/WSCALE

N_WARMUP_MM = 8     # pre-context garbage matmuls, N=256 each (~213ns cold):
                    # bridge prologue end (~7.1us) to first-data (~8.7us).
                    # Real matmuls then run cold-but-gapless until the HAM
                    # clock-gate un-throttles (~prologue+3.4us) — the gapless
                    # chain from the first warmup is what lets HAM fire; any
                    # pre-warm idle gap would reset its busy window.


@functools.lru_cache(maxsize=4)
def _build(t_pad: int):
    """Build + compile the per-core GEMM graph for token-pad t_pad."""
    mt = t_pad // P  # m tiles of 128 tokens

    nc = bacc.Bacc("TRN2", target_bir_lowering=False, debug=False)

    # xt[mi, p, ko, j] = X[mi*P + j, ko*P + p]  (bf16, used for ko 4..15)
    xt_d = nc.dram_tensor(
        "xt", [mt, P, KO, P], X_DT, kind="ExternalInput"
    ).ap()
    # xt8: same swizzle, e3m4, quarter 0 only (ko 0..3)
    xt8_d = nc.dram_tensor(
        "xt8", [mt, P, 4, P], W_DT, kind="ExternalInput"
    ).ap()
    # w[p, ko*N + n] = 64 * W[ko*P + p, n]  (full k-rows, k-major)
    w_d = nc.dram_tensor("w", [P, KO * N], W_DT, kind="ExternalInput").ap()
    out_d = nc.dram_tensor("out", [t_pad, N], OUT_DT, kind="ExternalOutput").ap()

    fast = mt <= 4  # phase-A needs 2 psum banks per m-tile

    # --- PE warm-up: raw pre-context matmuls on garbage SBUF, no deps.
    # They execute right after the engine prologue (~7.1us), before any
    # tile-scheduled instruction, keeping the PE busy until real data
    # arrives so the HAM clock-gate warms early. Results land in the
    # psum bank the tile pool will reuse; PE-queue FIFO order makes that
    # safe, and the first real accumulation starts with start=True.
    with (
        nc.sbuf_tensor("wu_raw", [P, 256], X_DT) as wu_h,
        nc.psum_tensor("wups_raw", [P, 512], mybir.dt.float32) as wps_h,
    ):
        wu = wu_h.ap()
        wps = wps_h.ap()
        for _ in range(N_WARMUP_MM):
            nc.tensor.matmul(wps[:, 0:256], wu[:, 0:P], wu[:],
                             start=True, stop=True, skip_group_check=True)

    with tile.TileContext(nc) as tc:
        with (
            tc.tile_pool(name="xt_p", bufs=1) as xt_p,
            tc.tile_pool(name="w_p", bufs=1) as w_p,
            tc.tile_pool(name="st_p", bufs=1) as st_p,
            tc.tile_pool(name="ps", bufs=8, space="PSUM") as ps,
        ):
            # phase-A psum tiles, allocated in the order their banks are
            # freed (casts fire per-m after its ko=15 matmuls) so phase-B's
            # pool cycling lines up with the frees
            psA = {}
            if fast:
                for m in range(mt):
                    for h in range(2):
                        psA[(m, h)] = ps.tile([P, 512], mybir.dt.float32,
                                              tag="psum", name=f"psA_{m}_{h}")

            # --- input DMAs in exact first-need order ---
            xt_t = {}           # (mi, quarter) -> (tile, base)
            w_t = {}            # ko -> (tile, sub-index)

            def dma_items():
                # round ko consumes xt quarter ko//4 (per m) + w row ko.
                # w rows 0-3 are split L/R: phase A only reads the left
                # 1024 cols, so the first rounds need 128KB/row and the
                # whole first-need window is ~1MB; the right halves are
                # only read in phase B (~28us later than their row's
                # phase-A use) and ride at the very end.
                for mi in range(mt):
                    yield ("xtq", (mi, 0))
                    if mi < 4:
                        yield ("wl", (mi,))  # w0L..w3L interleave with xt q0
                for c in range(min(mt, 4), 4):
                    yield ("wl", (c,))
                yield ("w", (4, 1))
                for mi in range(mt):
                    yield ("xtq", (mi, 1))
                yield ("w", (5, 1))
                yield ("w", (6, 1))
                yield ("w", (7, 1))
                for mi in range(mt):
                    yield ("xth", (mi,))   # quarters 2+3 as one [P,8,P]
                yield ("w", (8, 2))
                yield ("w", (10, 2))
                yield ("w", (12, 2))
                yield ("w", (14, 2))
                for c in range(4):
                    yield ("wr", (c,))

            # head items alternate the two HWDGE rings (parallel first
            # arrivals); the bulk rides sync so the scalar ring is free
            # early and Scalar only does output DMA issues later
            queues = [nc.sync, nc.scalar]
            qi = 0
            for idx, (kind, key) in enumerate(dma_items()):
                if idx < 2 * mt:
                    eng = queues[qi]
                    qi ^= 1
                else:
                    eng = nc.sync
                if kind == "xtq":
                    mi, q4 = key
                    # quarter 0 is e3m4 (64KB -> small first-need window);
                    # later quarters are bf16
                    dt = W_DT if q4 == 0 else X_DT
                    t = xt_p.tile([P, 4, P], dt, tag=f"xt_m{mi}q{q4}",
                                  name=f"xt_m{mi}q{q4}")
                    if q4 == 0:
                        eng.dma_start(t[:], xt8_d[mi, :, :, :])
                    else:
                        eng.dma_start(t[:], xt_d[mi, :, q4 * 4:(q4 + 1) * 4, :])
                    xt_t[(mi, q4)] = (t, 0)
                elif kind == "xth":
                    (mi,) = key
                    t = xt_p.tile([P, 8, P], X_DT, tag=f"xt_m{mi}h",
                                  name=f"xt_m{mi}h")
                    eng.dma_start(t[:], xt_d[mi, :, 8:16, :])
                    xt_t[(mi, 2)] = (t, 0)
                    xt_t[(mi, 3)] = (t, 4)
                elif kind in ("wl", "wr"):
                    (ko0,) = key
                    half = 0 if kind == "wl" else 1
                    t = w_p.tile([P, HALF], W_DT, tag=f"{kind}_{ko0}",
                                 name=f"{kind}_{ko0}")
                    o0 = ko0 * N + half * HALF
                    eng.dma_start(t[:], w_d[:, o0:o0 + HALF])
                    w_t[(ko0, half)] = (t, None)
                else:
                    ko0, nk = key
                    t = w_p.tile([P, nk, N], W_DT, tag=f"w_{ko0}",
                                 name=f"w_{ko0}")
                    eng.dma_start(t[:], w_d[:, ko0 * N:(ko0 + nk) * N])
                    for j in range(nk):
                        w_t[ko0 + j] = (t, j)

            def xt_ap(mi, ko):
                t, base = xt_t[(mi, ko // 4)]
                return t[:, base + (ko % 4), :]

            def w_ap(ko, j0, wdt=512):
                if ko < 4:
                    t, _ = w_t[(ko, j0 // HALF)]
                    return t[:, j0 % HALF:j0 % HALF + wdt]
                t, j = w_t[ko]
                return t[:, j, j0:j0 + wdt]

            if fast:
                # output staging: per (m, half) [P, 1024] bf16; DVE does
                # the casts, Scalar issues the output DMAs
                st = {(m, h): st_p.tile([P, HALF], OUT_DT, tag=f"st_{m}_{h}",
                                        name=f"st_{m}_{h}")
                      for m in range(mt) for h in range(2)}
                # --- phase A: left 1024 columns, ko-outer m-inner rounds.
                for ko in range(KO):
                    for m in range(mt):
                        lhsT = xt_ap(m, ko)
                        for h in range(2):
                            nc.tensor.matmul(
                                psA[(m, h)][:], lhsT, w_ap(ko, h * 512),
                                start=(ko == 0), stop=(ko == KO - 1),
                            )
                        if ko == KO - 1:
                            nc.vector.tensor_scalar_mul(
                                st[(m, 0)][:, 0:512], psA[(m, 0)][:],
                                1.0 / WSCALE)
                            nc.vector.tensor_scalar_mul(
                                st[(m, 0)][:, 512:1024], psA[(m, 1)][:],
                                1.0 / WSCALE)
                            nc.scalar.dma_start(
                                out_d[m * P:(m + 1) * P, 0:HALF],
                                st[(m, 0)][:])

                # --- phase B: right 1024 columns, m-major; w already in
                # SBUF (full rows), so no new input data is needed ---
                for m in range(mt):
                    last = m == mt - 1
                    lim = 1 if last else 2
                    psB = {}
                    for h in range(lim):
                        psB[h] = ps.tile([P, 512], mybir.dt.float32,
                                         tag="psum", name=f"psB_{m}_{h}")
                    for k in range(KO):
                        lhsT = xt_ap(m, k)
                        for h in range(lim):
                            nc.tensor.matmul(
                                psB[h][:], lhsT, w_ap(k, HALF + h * 512),
                                start=(k == 0), stop=(k == KO - 1),
                            )
                    for h in range(lim):
                        nc.vector.tensor_scalar_mul(
                            st[(m, 1)][:, h * 512:(h + 1) * 512], psB[h][:],
                            1.0 / WSCALE)
                    if not last:
                        nc.scalar.dma_start(
                            out_d[m * P:(m + 1) * P, HALF:N], st[(m, 1)][:])
                        continue
                    # last m-tile: cols 1536:2048 run as a 384-wide + a
                    # 128-wide accumulation group; cols 1024:1920 ship as
                    # ONE DMA after the 384 group's cast (fewer trailing
                    # completions), and only the tiny 128-col cast+DMA
                    # trails the final matmul
                    for g, (c0, cw) in enumerate(((512, 384), (896, 128))):
                        pg = ps.tile([P, cw], mybir.dt.float32,
                                     tag="psum", name=f"psum_l{g}")
                        for k in range(KO):
                            nc.tensor.matmul(
                                pg[:], xt_ap(m, k),
                                w_ap(k, HALF + c0, cw),
                                start=(k == 0), stop=(k == KO - 1),
                            )
                        nc.vector.tensor_scalar_mul(
                            st[(m, 1)][:, c0:c0 + cw], pg[:], 1.0 / WSCALE)
                        if g == 0:
                            nc.scalar.dma_start(
                                out_d[m * P:(m + 1) * P, HALF:HALF + 896],
                                st[(m, 1)][:, 0:896])
                        else:
                            nc.sync.dma_start(
                                out_d[m * P:(m + 1) * P,
                                      HALF + 896:N],
                                st[(m, 1)][:, 896:1024])
            else:
                # generic fallback (mt > 4): m-major over four 512-blocks
                for bi in range(4):
                    for m in range(mt):
                        psum_t = ps.tile([P, 512], mybir.dt.float32,
                                         tag="psum", name=f"ps_{bi}_{m}")
                        for k in range(KO):
                            nc.tensor.matmul(
                                psum_t[:], xt_ap(m, k), w_ap(k, bi * 512),
                                start=(k == 0), stop=(k == KO - 1),
                            )
                        o_sb = st_p.tile([P, 512], OUT_DT,
                                         tag=f"o{(bi * mt + m) % 4}",
                                         name=f"o_{bi}_{m}")
                        nc.vector.tensor_scalar_mul(o_sb[:], psum_t[:],
                                                    1.0 / WSCALE)
                        nc.scalar.dma_start(
                            out_d[m * P:(m + 1) * P,
                                  bi * 512:(bi + 1) * 512], o_sb[:])

    nc.compile()
    return nc


def _swizzle_x(x_pad: np.ndarray, t_pad: int):
    # [t_pad, K] f32 -> xt [mt, P, KO, P] bf16 + xt8 [mt, P, 4, P] e3m4,
    # xt[mi,p,ko,j] = X[mi*P+j, ko*P+p] (natural scale)
    mt = t_pad // P
    v = x_pad.reshape(mt, P, KO, P).transpose(0, 3, 2, 1)
    xt = np.ascontiguousarray(v.astype(NP_X))
    xt8 = np.ascontiguousarray(v[:, :, 0:4, :].astype(NP_W))
    return xt, xt8


def _pack_w(w_g: np.ndarray) -> np.ndarray:
    # [K, N] f32 -> [P, KO*N] e3m4 (scaled by WSCALE), k-major full rows:
    # w[p, ko*N + n] = 64 * W[ko*P+p, n]; each per-ko DMA chunk is one
    # contiguous 2048B run per partition line
    v = (w_g * WSCALE).reshape(KO, P, N).transpose(1, 0, 2).reshape(P, KO * N)
    return np.ascontiguousarray(v.astype(NP_W))


def _run(input, weight, tokens_per_expert, trace=False, **trace_kwargs):
    inp = np.ascontiguousarray(np.asarray(input), dtype=np.float32)
    wgt = np.ascontiguousarray(np.asarray(weight), dtype=np.float32)
    counts = np.asarray(tokens_per_expert).astype(np.int64)
    num_tokens, k = inp.shape
    assert k == K and wgt.shape == (G, K, N)
    # token group boundaries (matches searchsorted(cumsum, arange, 'right')),
    # clamped to the token range for safety on degenerate counts
    ends = np.minimum(np.cumsum(counts), num_tokens)
    starts = np.minimum(ends - counts, num_tokens)
    sizes = np.maximum(ends - starts, 0)

    t_pad = max(P, int(-(-max(int(sizes.max()), 1) // P)) * P)
    nc = _build(t_pad)

    in_maps = []
    for g in range(G):
        x_pad = np.zeros((t_pad, K), dtype=np.float32)
        x_pad[: sizes[g]] = inp[starts[g]:ends[g]]
        xt, xt8 = _swizzle_x(x_pad, t_pad)
        in_maps.append({"xt": xt, "xt8": xt8, "w": _pack_w(wgt[g])})

    res = bass_utils.run_bass_kernel_spmd(
        nc, in_maps, core_ids=list(range(G)), trace=trace, **trace_kwargs
    )

    # tokens not covered by any expert group get zero output (matches the
    # reference's masked accumulation)
    out = np.zeros((num_tokens, N), dtype=np.float32)
    for g in range(G):
        out[starts[g]:ends[g]] = res.results[g]["out"][: sizes[g]].astype(np.float32)
    return out, res


def kernel(input, weight, tokens_per_expert):
    out, _ = _run(input, weight, tokens_per_expert)
    return out


# revision 16
# speedup vs baseline: 1.0121x; 1.0121x over previous
"""AriaGroupedGEMM (MoE grouped GEMM) on 8 TRN2 NeuronCores.

Problem: input [4096, 2048] f32, weight [8, 2048, 2048] f32,
tokens_per_expert [8] int32 (tokens pre-sorted by expert).
out[i] = input[i] @ weight[expert_of(i)].

Strategy: expert-parallel. Core g owns expert g's weight and its token
group and runs a dense [T_pad, 2048] @ [2048, 2048] GEMM.

Precision: w is fp8 e3m4 pre-scaled by 64 on host (lifts ~N(0,0.02)
into e3m4's normal range); x is bf16 at natural scale for k 512..2047
and fp8 e3m4 (natural scale) for k 0..511, so every product is 64*x*w
and the PSUM->SBUF cast multiplies by 1/64. The PE's fp22 datapath
reproduces e3m4/bf16 products exactly (verified on HW vs exact
quantized arithmetic: 2e-8). Norm rel-err ~1.51e-2, under the 2e-2
gate. fp8 weights + fp8 first-quarter x shrink the first-need DMA
window to ~768KB and halve total weight traffic, so real matmuls can
start cold at ~8.7us and never starve.

Schedule (trace-driven):
- ~7.1us fixed engine-init preamble (runtime start gate + instruction
  load + prologue barrier) precedes everything; nothing can issue before
  it ends.
- Pre-TileContext warmup matmuls (raw, dep-free, garbage SBUF inputs)
  start right at the preamble end and keep the PE busy until real data
  lands, so the HAM clock-gate un-throttles to 2.4GHz ~3.4us later and
  real matmuls never run at the 1.2GHz cold rate for long.
- Input DMAs are issued in exact first-need order, the first 8
  alternating the two HWDGE rings (sync + scalar), the bulk on sync.
  w tiles are FULL k-rows [P, 2048] so phase B (right half) needs no
  additional DMA data at all.
- Phase A: left 1024 columns for all m-tiles, ko-outer m-inner (each
  fresh w row amortizes across 4 m-tiles -> compute-bound after round
  0). All 8 PSUM banks: 4 m x 2 halves. Per-m casts fire right after
  that m's ko=15 matmuls.
- Phase B: right 1024 columns, m-outer, reusing w from SBUF. The final
  m-tile computes its last 512 columns as two 256-wide groups so only a
  tiny cast+DMA trails the last matmul.
- All PSUM->SBUF casts run on the Vector engine (DVE does 16-bit output
  at 2 elem/cycle and never collides with the Scalar engine's DMA issue
  stream); output DMAs are issued from Scalar, the final one from Sync.
"""
import sys
import functools

for _p in ("/opt/trn_rl_repo", "/root/.axon_site/_ro/trn_rl_repo"):
    if _p not in sys.path:
        sys.path.insert(0, _p)

import numpy as np
import ml_dtypes

import concourse.mybir as mybir
import concourse.tile as tile
from concourse import bacc
from concourse import bass_utils

P = 128
K = 2048            # in_features (contraction)
N = 2048            # out_features
G = 8               # experts == cores
KO = K // P         # 16 k-subtiles
HALF = N // 2

X_DT = mybir.dt.bfloat16
W_DT = mybir.dt.float8e3
OUT_DT = mybir.dt.bfloat16      # psum(f32) -> bf16 on the way out; host upcasts
NP_X = ml_dtypes.bfloat16
NP_W = ml_dtypes.float8_e3m4

WSCALE = 64.0       # w*64 into e3m4; x stays natural scale; every psum
                    # value is 64*(x@w) and the cast scales by 1/WSCALE

N_WARMUP_MM = 16    # pre-context garbage matmuls, N=256 each (~213ns cold):
                    # bridge prologue end (~7.1us) to first-data-complete
                    # (~10.7us: last byte ~9 + DMA completion receipt ~1us).
                    # The HAM clock-gate needs a GAPLESS busy stretch of
                    # ~3.4us to un-throttle; 16 back-to-back warmups provide
                    # it, so real matmuls start at 2.4GHz with no pre-warm
                    # idle gap (a gap would reset the HAM busy window).


@functools.lru_cache(maxsize=4)
def _build(t_pad: int):
    """Build + compile the per-core GEMM graph for token-pad t_pad."""
    mt = t_pad // P  # m tiles of 128 tokens

    nc = bacc.Bacc("TRN2", target_bir_lowering=False, debug=False)

    # xt[mi, p, ko, j] = X[mi*P + j, ko*P + p]  (bf16, used for ko 4..15)
    xt_d = nc.dram_tensor(
        "xt", [mt, P, KO, P], X_DT, kind="ExternalInput"
    ).ap()
    # xt8: same swizzle, e3m4, quarter 0 only (ko 0..3)
    xt8_d = nc.dram_tensor(
        "xt8", [mt, P, 4, P], W_DT, kind="ExternalInput"
    ).ap()
    # w[p, ko*N + n] = 64 * W[ko*P + p, n]  (full k-rows, k-major)
    w_d = nc.dram_tensor("w", [P, KO * N], W_DT, kind="ExternalInput").ap()
    out_d = nc.dram_tensor("out", [t_pad, N], OUT_DT, kind="ExternalOutput").ap()

    fast = mt <= 4  # phase-A needs 2 psum banks per m-tile

    # --- PE warm-up: raw pre-context matmuls on garbage SBUF, no deps.
    # They execute right after the engine prologue (~7.1us), before any
    # tile-scheduled instruction, keeping the PE busy until real data
    # arrives so the HAM clock-gate warms early. Results land in the
    # psum bank the tile pool will reuse; PE-queue FIFO order makes that
    # safe, and the first real accumulation starts with start=True.
    with (
        nc.sbuf_tensor("wu_raw", [P, 256], X_DT) as wu_h,
        nc.psum_tensor("wups_raw", [P, 512], mybir.dt.float32) as wps_h,
    ):
        wu = wu_h.ap()
        wps = wps_h.ap()
        for _ in range(N_WARMUP_MM):
            nc.tensor.matmul(wps[:, 0:256], wu[:, 0:P], wu[:],
                             start=True, stop=True, skip_group_check=True)

    with tile.TileContext(nc) as tc:
        with (
            tc.tile_pool(name="xt_p", bufs=1) as xt_p,
            tc.tile_pool(name="w_p", bufs=1) as w_p,
            tc.tile_pool(name="st_p", bufs=1) as st_p,
            tc.tile_pool(name="ps", bufs=8, space="PSUM") as ps,
        ):
            # phase-A psum tiles, allocated in the order their banks are
            # freed (casts fire per-m after its ko=15 matmuls) so phase-B's
            # pool cycling lines up with the frees
            psA = {}
            if fast:
                for m in range(mt):
                    for h in range(2):
                        psA[(m, h)] = ps.tile([P, 512], mybir.dt.float32,
                                              tag="psum", name=f"psA_{m}_{h}")

            # --- input DMAs in exact first-need order ---
            xt_t = {}           # (mi, quarter) -> (tile, base)
            w_t = {}            # ko -> (tile, sub-index)

            def dma_items():
                # round ko consumes xt quarter ko//4 (per m) + w row ko.
                # w rows 0-3 are split L/R: phase A only reads the left
                # 1024 cols, so the first rounds need 128KB/row and the
                # whole first-need window is ~1MB; the right halves are
                # only read in phase B (~28us later than their row's
                # phase-A use) and ride at the very end.
                for mi in range(mt):
                    yield ("xtq", (mi, 0))
                    if mi < 4:
                        yield ("wl", (mi,))  # w0L..w3L interleave with xt q0
                for c in range(min(mt, 4), 4):
                    yield ("wl", (c,))
                yield ("w", (4, 1))
                for mi in range(mt):
                    yield ("xtq", (mi, 1))
                yield ("w", (5, 1))
                yield ("w", (6, 1))
                yield ("w", (7, 1))
                for mi in range(mt):
                    yield ("xth", (mi,))   # quarters 2+3 as one [P,8,P]
                yield ("w", (8, 2))
                yield ("w", (10, 2))
                yield ("w", (12, 2))
                yield ("w", (14, 2))
                for c in range(4):
                    yield ("wr", (c,))

            # head items alternate the two HWDGE rings (parallel first
            # arrivals); the bulk rides sync so the scalar ring is free
            # early and Scalar only does output DMA issues later
            queues = [nc.sync, nc.scalar]
            qi = 0
            for idx, (kind, key) in enumerate(dma_items()):
                if idx < 2 * mt:
                    eng = queues[qi]
                    qi ^= 1
                else:
                    eng = nc.sync
                if kind == "xtq":
                    mi, q4 = key
                    # quarter 0 is e3m4 (64KB -> small first-need window);
                    # later quarters are bf16
                    dt = W_DT if q4 == 0 else X_DT
                    t = xt_p.tile([P, 4, P], dt, tag=f"xt_m{mi}q{q4}",
                                  name=f"xt_m{mi}q{q4}")
                    if q4 == 0:
                        eng.dma_start(t[:], xt8_d[mi, :, :, :])
                    else:
                        eng.dma_start(t[:], xt_d[mi, :, q4 * 4:(q4 + 1) * 4, :])
                    xt_t[(mi, q4)] = (t, 0)
                elif kind == "xth":
                    (mi,) = key
                    t = xt_p.tile([P, 8, P], X_DT, tag=f"xt_m{mi}h",
                                  name=f"xt_m{mi}h")
                    eng.dma_start(t[:], xt_d[mi, :, 8:16, :])
                    xt_t[(mi, 2)] = (t, 0)
                    xt_t[(mi, 3)] = (t, 4)
                elif kind in ("wl", "wr"):
                    (ko0,) = key
                    half = 0 if kind == "wl" else 1
                    t = w_p.tile([P, HALF], W_DT, tag=f"{kind}_{ko0}",
                                 name=f"{kind}_{ko0}")
                    o0 = ko0 * N + half * HALF
                    eng.dma_start(t[:], w_d[:, o0:o0 + HALF])
                    w_t[(ko0, half)] = (t, None)
                else:
                    ko0, nk = key
                    t = w_p.tile([P, nk, N], W_DT, tag=f"w_{ko0}",
                                 name=f"w_{ko0}")
                    eng.dma_start(t[:], w_d[:, ko0 * N:(ko0 + nk) * N])
                    for j in range(nk):
                        w_t[ko0 + j] = (t, j)

            def xt_ap(mi, ko):
                t, base = xt_t[(mi, ko // 4)]
                return t[:, base + (ko % 4), :]

            def w_ap(ko, j0, wdt=512):
                if ko < 4:
                    t, _ = w_t[(ko, j0 // HALF)]
                    return t[:, j0 % HALF:j0 % HALF + wdt]
                t, j = w_t[ko]
                return t[:, j, j0:j0 + wdt]

            if fast:
                # output staging: per (m, half) [P, 1024] bf16; DVE does
                # the casts, Scalar issues the output DMAs
                st = {(m, h): st_p.tile([P, HALF], OUT_DT, tag=f"st_{m}_{h}",
                                        name=f"st_{m}_{h}")
                      for m in range(mt) for h in range(2)}
                # --- phase A: left 1024 columns, ko-outer m-inner rounds.
                for ko in range(KO):
                    for m in range(mt):
                        lhsT = xt_ap(m, ko)
                        for h in range(2):
                            nc.tensor.matmul(
                                psA[(m, h)][:], lhsT, w_ap(ko, h * 512),
                                start=(ko == 0), stop=(ko == KO - 1),
                            )
                        if ko == KO - 1:
                            nc.vector.tensor_scalar_mul(
                                st[(m, 0)][:, 0:512], psA[(m, 0)][:],
                                1.0 / WSCALE)
                            nc.vector.tensor_scalar_mul(
                                st[(m, 0)][:, 512:1024], psA[(m, 1)][:],
                                1.0 / WSCALE)
                            nc.scalar.dma_start(
                                out_d[m * P:(m + 1) * P, 0:HALF],
                                st[(m, 0)][:])

                # --- phase B: right 1024 columns, m-major; w already in
                # SBUF (full rows), so no new input data is needed ---
                for m in range(mt):
                    last = m == mt - 1
                    lim = 1 if last else 2
                    psB = {}
                    for h in range(lim):
                        psB[h] = ps.tile([P, 512], mybir.dt.float32,
                                         tag="psum", name=f"psB_{m}_{h}")
                    for k in range(KO):
                        lhsT = xt_ap(m, k)
                        for h in range(lim):
                            nc.tensor.matmul(
                                psB[h][:], lhsT, w_ap(k, HALF + h * 512),
                                start=(k == 0), stop=(k == KO - 1),
                            )
                    for h in range(lim):
                        nc.vector.tensor_scalar_mul(
                            st[(m, 1)][:, h * 512:(h + 1) * 512], psB[h][:],
                            1.0 / WSCALE)
                    if not last:
                        nc.scalar.dma_start(
                            out_d[m * P:(m + 1) * P, HALF:N], st[(m, 1)][:])
                        continue
                    # last m-tile: cols 1536:2048 run as a 384-wide + a
                    # 128-wide accumulation group; cols 1024:1920 ship as
                    # ONE DMA after the 384 group's cast (fewer trailing
                    # completions), and only the tiny 128-col cast+DMA
                    # trails the final matmul
                    for g, (c0, cw) in enumerate(((512, 384), (896, 128))):
                        pg = ps.tile([P, cw], mybir.dt.float32,
                                     tag="psum", name=f"psum_l{g}")
                        for k in range(KO):
                            nc.tensor.matmul(
                                pg[:], xt_ap(m, k),
                                w_ap(k, HALF + c0, cw),
                                start=(k == 0), stop=(k == KO - 1),
                            )
                        nc.vector.tensor_scalar_mul(
                            st[(m, 1)][:, c0:c0 + cw], pg[:], 1.0 / WSCALE)
                        if g == 0:
                            nc.scalar.dma_start(
                                out_d[m * P:(m + 1) * P, HALF:HALF + 896],
                                st[(m, 1)][:, 0:896])
                        else:
                            nc.sync.dma_start(
                                out_d[m * P:(m + 1) * P,
                                      HALF + 896:N],
                                st[(m, 1)][:, 896:1024])
            else:
                # generic fallback (mt > 4): m-major over four 512-blocks
                for bi in range(4):
                    for m in range(mt):
                        psum_t = ps.tile([P, 512], mybir.dt.float32,
                                         tag="psum", name=f"ps_{bi}_{m}")
                        for k in range(KO):
                            nc.tensor.matmul(
                                psum_t[:], xt_ap(m, k), w_ap(k, bi * 512),
                                start=(k == 0), stop=(k == KO - 1),
                            )
                        o_sb = st_p.tile([P, 512], OUT_DT,
                                         tag=f"o{(bi * mt + m) % 4}",
                                         name=f"o_{bi}_{m}")
                        nc.vector.tensor_scalar_mul(o_sb[:], psum_t[:],
                                                    1.0 / WSCALE)
                        nc.scalar.dma_start(
                            out_d[m * P:(m + 1) * P,
                                  bi * 512:(bi + 1) * 512], o_sb[:])

    nc.compile()
    return nc


def _swizzle_x(x_pad: np.ndarray, t_pad: int):
    # [t_pad, K] f32 -> xt [mt, P, KO, P] bf16 + xt8 [mt, P, 4, P] e3m4,
    # xt[mi,p,ko,j] = X[mi*P+j, ko*P+p] (natural scale)
    mt = t_pad // P
    v = x_pad.reshape(mt, P, KO, P).transpose(0, 3, 2, 1)
    xt = np.ascontiguousarray(v.astype(NP_X))
    xt8 = np.ascontiguousarray(v[:, :, 0:4, :].astype(NP_W))
    return xt, xt8


def _pack_w(w_g: np.ndarray) -> np.ndarray:
    # [K, N] f32 -> [P, KO*N] e3m4 (scaled by WSCALE), k-major full rows:
    # w[p, ko*N + n] = 64 * W[ko*P+p, n]; each per-ko DMA chunk is one
    # contiguous 2048B run per partition line
    v = (w_g * WSCALE).reshape(KO, P, N).transpose(1, 0, 2).reshape(P, KO * N)
    return np.ascontiguousarray(v.astype(NP_W))


def _run(input, weight, tokens_per_expert, trace=False, **trace_kwargs):
    inp = np.ascontiguousarray(np.asarray(input), dtype=np.float32)
    wgt = np.ascontiguousarray(np.asarray(weight), dtype=np.float32)
    counts = np.asarray(tokens_per_expert).astype(np.int64)
    num_tokens, k = inp.shape
    assert k == K and wgt.shape == (G, K, N)
    # token group boundaries (matches searchsorted(cumsum, arange, 'right')),
    # clamped to the token range for safety on degenerate counts
    ends = np.minimum(np.cumsum(counts), num_tokens)
    starts = np.minimum(ends - counts, num_tokens)
    sizes = np.maximum(ends - starts, 0)

    t_pad = max(P, int(-(-max(int(sizes.max()), 1) // P)) * P)
    nc = _build(t_pad)

    in_maps = []
    for g in range(G):
        x_pad = np.zeros((t_pad, K), dtype=np.float32)
        x_pad[: sizes[g]] = inp[starts[g]:ends[g]]
        xt, xt8 = _swizzle_x(x_pad, t_pad)
        in_maps.append({"xt": xt, "xt8": xt8, "w": _pack_w(wgt[g])})

    res = bass_utils.run_bass_kernel_spmd(
        nc, in_maps, core_ids=list(range(G)), trace=trace, **trace_kwargs
    )

    # tokens not covered by any expert group get zero output (matches the
    # reference's masked accumulation)
    out = np.zeros((num_tokens, N), dtype=np.float32)
    for g in range(G):
        out[starts[g]:ends[g]] = res.results[g]["out"][: sizes[g]].astype(np.float32)
    return out, res


def kernel(input, weight, tokens_per_expert):
    out, _ = _run(input, weight, tokens_per_expert)
    return out


# revision 17
# speedup vs baseline: 1.0134x; 1.0013x over previous
"""AriaGroupedGEMM (MoE grouped GEMM) on 8 TRN2 NeuronCores.

Problem: input [4096, 2048] f32, weight [8, 2048, 2048] f32,
tokens_per_expert [8] int32 (tokens pre-sorted by expert).
out[i] = input[i] @ weight[expert_of(i)].

Strategy: expert-parallel. Core g owns expert g's weight and its token
group and runs a dense [T_pad, 2048] @ [2048, 2048] GEMM.

Precision: w is fp8 e3m4 pre-scaled by 64 on host (lifts ~N(0,0.02)
into e3m4's normal range); x is bf16 at natural scale for k 512..2047
and fp8 e3m4 (natural scale) for k 0..511, so every product is 64*x*w
and the PSUM->SBUF cast multiplies by 1/64. The PE's fp22 datapath
reproduces e3m4/bf16 products exactly (verified on HW vs exact
quantized arithmetic: 2e-8). Norm rel-err ~1.51e-2, under the 2e-2
gate. fp8 weights + fp8 first-quarter x shrink the first-need DMA
window to ~768KB and halve total weight traffic, so real matmuls can
start cold at ~8.7us and never starve.

Schedule (trace-driven):
- ~7.1us fixed engine-init preamble (runtime start gate + instruction
  load + prologue barrier) precedes everything; nothing can issue before
  it ends.
- Pre-TileContext warmup matmuls (raw, dep-free, garbage SBUF inputs)
  start right at the preamble end and keep the PE busy until real data
  lands, so the HAM clock-gate un-throttles to 2.4GHz ~3.4us later and
  real matmuls never run at the 1.2GHz cold rate for long.
- Input DMAs are issued in exact first-need order, the first 8
  alternating the two HWDGE rings (sync + scalar), the bulk on sync.
  w tiles are FULL k-rows [P, 2048] so phase B (right half) needs no
  additional DMA data at all.
- Phase A: left 1024 columns for all m-tiles, ko-outer m-inner (each
  fresh w row amortizes across 4 m-tiles -> compute-bound after round
  0). All 8 PSUM banks: 4 m x 2 halves. Per-m casts fire right after
  that m's ko=15 matmuls.
- Phase B: right 1024 columns, m-outer, reusing w from SBUF. The final
  m-tile computes its last 512 columns as two 256-wide groups so only a
  tiny cast+DMA trails the last matmul.
- All PSUM->SBUF casts run on the Vector engine (DVE does 16-bit output
  at 2 elem/cycle and never collides with the Scalar engine's DMA issue
  stream); output DMAs are issued from Scalar, the final one from Sync.
"""
import sys
import functools

for _p in ("/opt/trn_rl_repo", "/root/.axon_site/_ro/trn_rl_repo"):
    if _p not in sys.path:
        sys.path.insert(0, _p)

import numpy as np
import ml_dtypes

import concourse.mybir as mybir
import concourse.tile as tile
from concourse import bacc
from concourse import bass_utils

P = 128
K = 2048            # in_features (contraction)
N = 2048            # out_features
G = 8               # experts == cores
KO = K // P         # 16 k-subtiles
HALF = N // 2

X_DT = mybir.dt.bfloat16
W_DT = mybir.dt.float8e3
OUT_DT = mybir.dt.bfloat16      # psum(f32) -> bf16 on the way out; host upcasts
NP_X = ml_dtypes.bfloat16
NP_W = ml_dtypes.float8_e3m4

WSCALE = 64.0       # w*64 into e3m4; x stays natural scale; every psum
                    # value is 64*(x@w) and the cast scales by 1/WSCALE

N_WARMUP_MM = 21    # pre-context garbage matmuls, N=256 each (~213ns cold):
                    # bridge prologue end (~7.1us) to first-data-complete
                    # (~10.7us: last byte ~9 + DMA completion receipt ~1us).
                    # The HAM clock-gate needs a GAPLESS busy stretch of
                    # ~3.4us to un-throttle; 16 back-to-back warmups provide
                    # it, so real matmuls start at 2.4GHz with no pre-warm
                    # idle gap (a gap would reset the HAM busy window).


@functools.lru_cache(maxsize=4)
def _build(t_pad: int):
    """Build + compile the per-core GEMM graph for token-pad t_pad."""
    mt = t_pad // P  # m tiles of 128 tokens

    nc = bacc.Bacc("TRN2", target_bir_lowering=False, debug=False)

    # xt[mi, p, ko, j] = X[mi*P + j, ko*P + p]  (bf16, used for ko 4..15)
    xt_d = nc.dram_tensor(
        "xt", [mt, P, KO, P], X_DT, kind="ExternalInput"
    ).ap()
    # xt8: same swizzle, e3m4, quarter 0 only (ko 0..3)
    xt8_d = nc.dram_tensor(
        "xt8", [mt, P, 4, P], W_DT, kind="ExternalInput"
    ).ap()
    # w[p, ko*N + n] = 64 * W[ko*P + p, n]  (full k-rows, k-major)
    w_d = nc.dram_tensor("w", [P, KO * N], W_DT, kind="ExternalInput").ap()
    out_d = nc.dram_tensor("out", [t_pad, N], OUT_DT, kind="ExternalOutput").ap()

    fast = mt <= 4  # phase-A needs 2 psum banks per m-tile

    # --- PE warm-up: raw pre-context matmuls on garbage SBUF, no deps.
    # They execute right after the engine prologue (~7.1us), before any
    # tile-scheduled instruction, keeping the PE busy until real data
    # arrives so the HAM clock-gate warms early. Results land in the
    # psum bank the tile pool will reuse; PE-queue FIFO order makes that
    # safe, and the first real accumulation starts with start=True.
    with (
        nc.sbuf_tensor("wu_raw", [P, 256], X_DT) as wu_h,
        nc.psum_tensor("wups_raw", [P, 512], mybir.dt.float32) as wps_h,
    ):
        wu = wu_h.ap()
        wps = wps_h.ap()
        for _ in range(N_WARMUP_MM):
            nc.tensor.matmul(wps[:, 0:256], wu[:, 0:P], wu[:],
                             start=True, stop=True, skip_group_check=True)

    with tile.TileContext(nc) as tc:
        with (
            tc.tile_pool(name="xt_p", bufs=1) as xt_p,
            tc.tile_pool(name="w_p", bufs=1) as w_p,
            tc.tile_pool(name="st_p", bufs=1) as st_p,
            tc.tile_pool(name="ps", bufs=8, space="PSUM") as ps,
        ):
            # phase-A psum tiles, allocated in the order their banks are
            # freed (casts fire per-m after its ko=15 matmuls) so phase-B's
            # pool cycling lines up with the frees
            psA = {}
            if fast:
                for m in range(mt):
                    for h in range(2):
                        psA[(m, h)] = ps.tile([P, 512], mybir.dt.float32,
                                              tag="psum", name=f"psA_{m}_{h}")

            # --- input DMAs in exact first-need order ---
            xt_t = {}           # (mi, quarter) -> (tile, base)
            w_t = {}            # ko -> (tile, sub-index)

            def dma_items():
                # round ko consumes xt quarter ko//4 (per m) + w row ko.
                # w rows 0-3 are split L/R: phase A only reads the left
                # 1024 cols, so the first rounds need 128KB/row and the
                # whole first-need window is ~1MB; the right halves are
                # only read in phase B (~28us later than their row's
                # phase-A use) and ride at the very end.
                for mi in range(mt):
                    yield ("xtq", (mi, 0))
                    if mi < 4:
                        yield ("wl", (mi,))  # w0L..w3L interleave with xt q0
                for c in range(min(mt, 4), 4):
                    yield ("wl", (c,))
                yield ("w", (4, 1))
                for mi in range(mt):
                    yield ("xtq", (mi, 1))
                yield ("w", (5, 1))
                yield ("w", (6, 1))
                yield ("w", (7, 1))
                for mi in range(mt):
                    yield ("xth", (mi,))   # quarters 2+3 as one [P,8,P]
                yield ("w", (8, 2))
                yield ("w", (10, 2))
                yield ("w", (12, 2))
                yield ("w", (14, 2))
                for c in range(4):
                    yield ("wr", (c,))

            # head items alternate the two HWDGE rings (parallel first
            # arrivals); the bulk rides sync so the scalar ring is free
            # early and Scalar only does output DMA issues later
            queues = [nc.sync, nc.scalar]
            qi = 0
            for idx, (kind, key) in enumerate(dma_items()):
                if idx < 2 * mt:
                    eng = queues[qi]
                    qi ^= 1
                else:
                    eng = nc.sync
                if kind == "xtq":
                    mi, q4 = key
                    # quarter 0 is e3m4 (64KB -> small first-need window);
                    # later quarters are bf16
                    dt = W_DT if q4 == 0 else X_DT
                    t = xt_p.tile([P, 4, P], dt, tag=f"xt_m{mi}q{q4}",
                                  name=f"xt_m{mi}q{q4}")
                    if q4 == 0:
                        eng.dma_start(t[:], xt8_d[mi, :, :, :])
                    else:
                        eng.dma_start(t[:], xt_d[mi, :, q4 * 4:(q4 + 1) * 4, :])
                    xt_t[(mi, q4)] = (t, 0)
                elif kind == "xth":
                    (mi,) = key
                    t = xt_p.tile([P, 8, P], X_DT, tag=f"xt_m{mi}h",
                                  name=f"xt_m{mi}h")
                    eng.dma_start(t[:], xt_d[mi, :, 8:16, :])
                    xt_t[(mi, 2)] = (t, 0)
                    xt_t[(mi, 3)] = (t, 4)
                elif kind in ("wl", "wr"):
                    (ko0,) = key
                    half = 0 if kind == "wl" else 1
                    t = w_p.tile([P, HALF], W_DT, tag=f"{kind}_{ko0}",
                                 name=f"{kind}_{ko0}")
                    o0 = ko0 * N + half * HALF
                    eng.dma_start(t[:], w_d[:, o0:o0 + HALF])
                    w_t[(ko0, half)] = (t, None)
                else:
                    ko0, nk = key
                    t = w_p.tile([P, nk, N], W_DT, tag=f"w_{ko0}",
                                 name=f"w_{ko0}")
                    eng.dma_start(t[:], w_d[:, ko0 * N:(ko0 + nk) * N])
                    for j in range(nk):
                        w_t[ko0 + j] = (t, j)

            def xt_ap(mi, ko):
                t, base = xt_t[(mi, ko // 4)]
                return t[:, base + (ko % 4), :]

            def w_ap(ko, j0, wdt=512):
                if ko < 4:
                    t, _ = w_t[(ko, j0 // HALF)]
                    return t[:, j0 % HALF:j0 % HALF + wdt]
                t, j = w_t[ko]
                return t[:, j, j0:j0 + wdt]

            if fast:
                # output staging: per (m, half) [P, 1024] bf16; DVE does
                # the casts, Scalar issues the output DMAs
                st = {(m, h): st_p.tile([P, HALF], OUT_DT, tag=f"st_{m}_{h}",
                                        name=f"st_{m}_{h}")
                      for m in range(mt) for h in range(2)}
                # --- phase A: left 1024 columns, ko-outer m-inner rounds.
                for ko in range(KO):
                    for m in range(mt):
                        lhsT = xt_ap(m, ko)
                        for h in range(2):
                            nc.tensor.matmul(
                                psA[(m, h)][:], lhsT, w_ap(ko, h * 512),
                                start=(ko == 0), stop=(ko == KO - 1),
                            )
                        if ko == KO - 1:
                            nc.vector.tensor_scalar_mul(
                                st[(m, 0)][:, 0:512], psA[(m, 0)][:],
                                1.0 / WSCALE)
                            nc.vector.tensor_scalar_mul(
                                st[(m, 0)][:, 512:1024], psA[(m, 1)][:],
                                1.0 / WSCALE)
                            nc.scalar.dma_start(
                                out_d[m * P:(m + 1) * P, 0:HALF],
                                st[(m, 0)][:])

                # --- phase B: right 1024 columns, m-major; w already in
                # SBUF (full rows), so no new input data is needed ---
                for m in range(mt):
                    last = m == mt - 1
                    lim = 1 if last else 2
                    psB = {}
                    for h in range(lim):
                        psB[h] = ps.tile([P, 512], mybir.dt.float32,
                                         tag="psum", name=f"psB_{m}_{h}")
                    for k in range(KO):
                        lhsT = xt_ap(m, k)
                        for h in range(lim):
                            nc.tensor.matmul(
                                psB[h][:], lhsT, w_ap(k, HALF + h * 512),
                                start=(k == 0), stop=(k == KO - 1),
                            )
                    for h in range(lim):
                        nc.vector.tensor_scalar_mul(
                            st[(m, 1)][:, h * 512:(h + 1) * 512], psB[h][:],
                            1.0 / WSCALE)
                    if not last:
                        nc.scalar.dma_start(
                            out_d[m * P:(m + 1) * P, HALF:N], st[(m, 1)][:])
                        continue
                    # last m-tile: cols 1536:2048 run as a 384-wide + a
                    # 128-wide accumulation group; cols 1024:1920 ship as
                    # ONE DMA after the 384 group's cast (fewer trailing
                    # completions), and only the tiny 128-col cast+DMA
                    # trails the final matmul
                    for g, (c0, cw) in enumerate(((512, 384), (896, 128))):
                        pg = ps.tile([P, cw], mybir.dt.float32,
                                     tag="psum", name=f"psum_l{g}")
                        for k in range(KO):
                            nc.tensor.matmul(
                                pg[:], xt_ap(m, k),
                                w_ap(k, HALF + c0, cw),
                                start=(k == 0), stop=(k == KO - 1),
                            )
                        nc.vector.tensor_scalar_mul(
                            st[(m, 1)][:, c0:c0 + cw], pg[:], 1.0 / WSCALE)
                        if g == 0:
                            nc.scalar.dma_start(
                                out_d[m * P:(m + 1) * P, HALF:HALF + 896],
                                st[(m, 1)][:, 0:896])
                        else:
                            nc.sync.dma_start(
                                out_d[m * P:(m + 1) * P,
                                      HALF + 896:N],
                                st[(m, 1)][:, 896:1024])
            else:
                # generic fallback (mt > 4): m-major over four 512-blocks
                for bi in range(4):
                    for m in range(mt):
                        psum_t = ps.tile([P, 512], mybir.dt.float32,
                                         tag="psum", name=f"ps_{bi}_{m}")
                        for k in range(KO):
                            nc.tensor.matmul(
                                psum_t[:], xt_ap(m, k), w_ap(k, bi * 512),
                                start=(k == 0), stop=(k == KO - 1),
                            )
                        o_sb = st_p.tile([P, 512], OUT_DT,
                                         tag=f"o{(bi * mt + m) % 4}",
                                         name=f"o_{bi}_{m}")
                        nc.vector.tensor_scalar_mul(o_sb[:], psum_t[:],
                                                    1.0 / WSCALE)
                        nc.scalar.dma_start(
                            out_d[m * P:(m + 1) * P,
                                  bi * 512:(bi + 1) * 512], o_sb[:])

    nc.compile()
    return nc


def _swizzle_x(x_pad: np.ndarray, t_pad: int):
    # [t_pad, K] f32 -> xt [mt, P, KO, P] bf16 + xt8 [mt, P, 4, P] e3m4,
    # xt[mi,p,ko,j] = X[mi*P+j, ko*P+p] (natural scale)
    mt = t_pad // P
    v = x_pad.reshape(mt, P, KO, P).transpose(0, 3, 2, 1)
    xt = np.ascontiguousarray(v.astype(NP_X))
    xt8 = np.ascontiguousarray(v[:, :, 0:4, :].astype(NP_W))
    return xt, xt8


def _pack_w(w_g: np.ndarray) -> np.ndarray:
    # [K, N] f32 -> [P, KO*N] e3m4 (scaled by WSCALE), k-major full rows:
    # w[p, ko*N + n] = 64 * W[ko*P+p, n]; each per-ko DMA chunk is one
    # contiguous 2048B run per partition line
    v = (w_g * WSCALE).reshape(KO, P, N).transpose(1, 0, 2).reshape(P, KO * N)
    return np.ascontiguousarray(v.astype(NP_W))


def _run(input, weight, tokens_per_expert, trace=False, **trace_kwargs):
    inp = np.ascontiguousarray(np.asarray(input), dtype=np.float32)
    wgt = np.ascontiguousarray(np.asarray(weight), dtype=np.float32)
    counts = np.asarray(tokens_per_expert).astype(np.int64)
    num_tokens, k = inp.shape
    assert k == K and wgt.shape == (G, K, N)
    # token group boundaries (matches searchsorted(cumsum, arange, 'right')),
    # clamped to the token range for safety on degenerate counts
    ends = np.minimum(np.cumsum(counts), num_tokens)
    starts = np.minimum(ends - counts, num_tokens)
    sizes = np.maximum(ends - starts, 0)

    t_pad = max(P, int(-(-max(int(sizes.max()), 1) // P)) * P)
    nc = _build(t_pad)

    in_maps = []
    for g in range(G):
        x_pad = np.zeros((t_pad, K), dtype=np.float32)
        x_pad[: sizes[g]] = inp[starts[g]:ends[g]]
        xt, xt8 = _swizzle_x(x_pad, t_pad)
        in_maps.append({"xt": xt, "xt8": xt8, "w": _pack_w(wgt[g])})

    res = bass_utils.run_bass_kernel_spmd(
        nc, in_maps, core_ids=list(range(G)), trace=trace, **trace_kwargs
    )

    # tokens not covered by any expert group get zero output (matches the
    # reference's masked accumulation)
    out = np.zeros((num_tokens, N), dtype=np.float32)
    for g in range(G):
        out[starts[g]:ends[g]] = res.results[g]["out"][: sizes[g]].astype(np.float32)
    return out, res


def kernel(input, weight, tokens_per_expert):
    out, _ = _run(input, weight, tokens_per_expert)
    return out
